# revision 1
# baseline (speedup 1.0000x reference)
"""Fused QKV projection + correlation attention (softmax over keys) on 8 trn2 cores.

Problem: x[4,2048,1024] f32; K/Q/V = x@W* + b*; out = softmax(Q Kt / 32, keys) @ V.

Sharding: core c -> batch b=c//2, key-half h=c%2.  Each core:
  - projects K,V for its 1024-key half, Q for all 2048 queries of its batch
  - computes U = exp(Q Kt/32) @ V  (unnormalized) and rs = rowsum(exp(..))
Host combines per-batch:  out[b] = (U0+U1)/(rs0+rs1)[:,None] + bv
(no max-subtraction needed: scores ~ N(0,1), exp stays within fp32 range).

Device layouts (partition dim first):
  xkvT [d, sk]   xqT [d, sq]  (host pre-transposed; projections contract over
  d on the partition axis, so x must appear transposed -- free on the host)
  KT[dout, sk], QT[dout, sq] from lhsT=W chunk;  V[sk, d] from lhsT=xkvT chunk
  scoresT[sk, sq] from lhsT=KT chunk, rhs=QT -> exp on ACT (scale=1/32 folded)
  U[sq, d] from lhsT=expT chunk, rhs=V;  rs via lhsT=ones[128,1], rhs=expT.

Matmul dtype `MM`: float32r (full PE rate at N=512, needs producers to round
-> DMA-loaded x/W pass through a DVE/ACT copy) or bfloat16 (host casts x/W).
"""

import numpy as np

B, S, D = 4, 2048, 1024
N_CORES = 8
MM = "float32r"  # "float32r" | "bfloat16" | "float32"

_BUILD_CACHE = {}
_RUN_KWARGS = {}      # test.py sets {"trace": True, ...} for profiling runs
_LAST_RESULTS = None  # BassKernelResults of the last run


def _build(d, sk, sq, mm=MM):
    """Build the per-core module. d: model dim; sk: keys/core; sq: queries/core."""
    key = (d, sk, sq, mm)
    if key in _BUILD_CACHE:
        return _BUILD_CACHE[key]

    from contextlib import ExitStack

    import concourse.bass as bass  # noqa: F401
    import concourse.mybir as mybir
    from concourse import bacc
    from concourse.tile import TileContext

    f32 = mybir.dt.float32
    mmdt = getattr(mybir.dt, mm)
    rounded = mm == "float32r"   # DMA-loaded operands need a rounding copy
    in_dt = mmdt if mm == "bfloat16" else f32  # dram dtype of x / W inputs

    P = 128
    NFREE = 512  # max fp32 moving free dim / one psum bank
    DC = d // P              # d chunks (contraction + dout chunks)
    KC = sk // P             # key chunks
    BLK = min(sq, NFREE)     # sq block width
    NBLK = sq // BLK
    SQ4 = BLK // P           # 128-row sq chunks per block
    NKB = max(1, sk // NFREE)
    KB = min(sk, NFREE)
    ND = max(1, d // NFREE)
    DB = min(d, NFREE)
    scale = float(1.0 / np.sqrt(np.float32(d)))

    nc = bacc.Bacc("TRN2", target_bir_lowering=False)
    Ident = mybir.ActivationFunctionType.Identity
    Exp = mybir.ActivationFunctionType.Exp

    xkvT = nc.dram_tensor("xkvT", [d, sk], in_dt, kind="ExternalInput")
    xqT = nc.dram_tensor("xqT", [d, sq], in_dt, kind="ExternalInput")
    Wk = nc.dram_tensor("Wk", [d, d], in_dt, kind="ExternalInput")
    Wq = nc.dram_tensor("Wq", [d, d], in_dt, kind="ExternalInput")
    Wv = nc.dram_tensor("Wv", [d, d], in_dt, kind="ExternalInput")
    bk = nc.dram_tensor("bk", [d], f32, kind="ExternalInput")
    bq = nc.dram_tensor("bq", [d], f32, kind="ExternalInput")
    U = nc.dram_tensor("U", [sq, d], f32, kind="ExternalOutput")
    rs = nc.dram_tensor("rs", [sq], f32, kind="ExternalOutput")

    xkvT_v = xkvT.ap().rearrange("(c p) s -> c p s", p=P)
    xqT_v = xqT.ap().rearrange("(c p) s -> c p s", p=P)
    Wk_v = Wk.ap().rearrange("(c p) e -> c p e", p=P)
    Wq_v = Wq.ap().rearrange("(c p) e -> p c e", p=P)  # [128, DC, d]
    Wv_v = Wv.ap().rearrange("(c p) e -> c p e", p=P)

    with TileContext(nc) as tc, ExitStack() as outer:
        resid = outer.enter_context(tc.tile_pool(name="resid", bufs=1))

        KT_sb = resid.tile([P, DC, sk], mmdt)     # [dout, sk]
        V_sb = resid.tile([P, KC, d], mmdt)       # [sk, d]
        bk_sb = resid.tile([P, DC], f32)
        bq_sb = resid.tile([P, DC], f32)
        ones_f = resid.tile([P, 1], f32)
        ones_sb = resid.tile([P, 1], mmdt)
        rs_stage = resid.tile([1, sq], f32)

        nc.vector.memset(ones_f, 1.0)
        nc.vector.tensor_copy(ones_sb, ones_f)
        nc.sync.dma_start(out=bk_sb, in_=bk.ap().rearrange("(c p) -> p c", p=P))
        nc.sync.dma_start(out=bq_sb, in_=bq.ap().rearrange("(c p) -> p c", p=P))

        def load(pool, stg_pool, dram_ap, shape, name, engine):
            """DMA dram -> mmdt tile, rounding through f32 staging if needed."""
            t = pool.tile([P, *shape], mmdt, name=name)
            if rounded:
                stg = stg_pool.tile([P, *shape], f32, name=f"{name}_stg")
                nc.sync.dma_start(out=stg, in_=dram_ap)
                engine(t, stg)
            else:
                nc.sync.dma_start(out=t, in_=dram_ap)
            return t

        # ---------------- stage 0: K and V projections (key half) ----------
        with ExitStack() as s0:
            p0 = s0.enter_context(tc.tile_pool(name="p0", bufs=1))
            stg0 = s0.enter_context(tc.tile_pool(name="stg0", bufs=6))
            ps0 = s0.enter_context(tc.tile_pool(name="ps0", bufs=4, space="PSUM"))

            xkv_sb = p0.tile([P, DC, sk], mmdt)
            Wk_sb = p0.tile([P, DC, d], mmdt)
            Wv_sb = p0.tile([P, DC, d], mmdt)
            # xkv+Wk first (K-proj needs them all); Wv only feeds V-proj later
            loads = [(xkv_sb, xkvT_v, nc.vector.tensor_copy),
                     (Wk_sb, Wk_v, nc.scalar.copy)]
            loads = [(t, v, e, c) for c in range(DC) for t, v, e in loads]
            loads += [(Wv_sb, Wv_v,
                       nc.vector.tensor_copy if c % 2 else nc.scalar.copy, c)
                      for c in range(DC)]
            for dst, src, eng, c in loads:
                if rounded:
                    stg = stg0.tile([P, max(sk, d)], f32, name="stg")
                    nc.sync.dma_start(out=stg[:, :src[c].shape[-1]], in_=src[c])
                    eng(dst[:, c, :], stg[:, :src[c].shape[-1]])
                else:
                    nc.sync.dma_start(out=dst[:, c, :], in_=src[c])

            # KT[dout m, sk] = sum_k Wk[k,m]^T xkv[k,:]   (+bk on evacuation)
            for m in range(DC):
                for nb in range(NKB):
                    ps = ps0.tile([P, KB], f32, name="ps_proj")
                    for k in range(DC):
                        nc.tensor.matmul(
                            ps,
                            Wk_sb[:, k, m * P:(m + 1) * P],
                            xkv_sb[:, k, nb * KB:(nb + 1) * KB],
                            start=(k == 0), stop=(k == DC - 1),
                        )
                    nc.scalar.activation(
                        KT_sb[:, m, nb * KB:(nb + 1) * KB], ps, Ident,
                        bias=bk_sb[:, m:m + 1], scale=1.0,
                    )
            # V[sk m, d] = sum_k xkv[k,m]^T Wv[k,:]   (bv added on host)
            for m in range(KC):
                for nb in range(ND):
                    ps = ps0.tile([P, DB], f32, name="ps_proj")
                    for k in range(DC):
                        nc.tensor.matmul(
                            ps,
                            xkv_sb[:, k, m * P:(m + 1) * P],
                            Wv_sb[:, k, nb * DB:(nb + 1) * DB],
                            start=(k == 0), stop=(k == DC - 1),
                        )
                    nc.vector.tensor_copy(V_sb[:, m, nb * DB:(nb + 1) * DB], ps)

        # ---------------- stage 1: per sq-block Q proj, scores, exp, AV ----
        with ExitStack() as s1:
            pwq = s1.enter_context(tc.tile_pool(name="pwq", bufs=3))
            stgq = s1.enter_context(tc.tile_pool(name="stgq", bufs=2))
            pxq = s1.enter_context(tc.tile_pool(name="pxq", bufs=2))
            pqt = s1.enter_context(tc.tile_pool(name="pqt", bufs=2))
            pexp = s1.enter_context(tc.tile_pool(name="pexp", bufs=2))
            pout = s1.enter_context(tc.tile_pool(name="pout", bufs=4))
            ps_sh = s1.enter_context(tc.tile_pool(name="ps_sh", bufs=4, space="PSUM"))
            ps_av = s1.enter_context(tc.tile_pool(name="ps_av", bufs=4, space="PSUM"))

            for blk in range(NBLK):
                lo = blk * BLK
                # Q projection inputs for this block of queries
                xq_blk = pxq.tile([P, DC, BLK], mmdt)
                for c in range(DC):
                    if rounded:
                        stg = stgq.tile([P, BLK], f32, name="stg_xq")
                        nc.sync.dma_start(out=stg, in_=xqT_v[c][:, lo:lo + BLK])
                        # alternate engines so neither DVE (V/AV evacs) nor
                        # ACT (QT/exp evacs) serializes the block start
                        (nc.vector.tensor_copy if c % 2 else nc.scalar.copy)(
                            xq_blk[:, c, :], stg)
                    else:
                        nc.sync.dma_start(
                            out=xq_blk[:, c, :], in_=xqT_v[c][:, lo:lo + BLK])
                qt_blk = pqt.tile([P, DC, BLK], mmdt)
                for m in range(DC):
                    wq_m = load(
                        pwq, stgq, Wq_v[:, :, m * P:(m + 1) * P],
                        [DC, P], "wq_m", nc.scalar.copy,
                    )
                    ps = ps_sh.tile([P, BLK], f32, name="ps_q", tag="ps_sh")
                    for k in range(DC):
                        nc.tensor.matmul(
                            ps, wq_m[:, k, :], xq_blk[:, k, :],
                            start=(k == 0), stop=(k == DC - 1),
                        )
                    nc.scalar.activation(
                        qt_blk[:, m, :], ps, Ident,
                        bias=bq_sb[:, m:m + 1], scale=1.0,
                    )
                # scoresT + exp:  expT[sk, sq_blk] = exp(scale * KT^T Q)
                exp_blk = pexp.tile([P, KC, BLK], mmdt)
                for skc in range(KC):
                    ps = ps_sh.tile([P, BLK], f32, name="ps_s", tag="ps_sh")
                    for dc in range(DC):
                        nc.tensor.matmul(
                            ps,
                            KT_sb[:, dc, skc * P:(skc + 1) * P],
                            qt_blk[:, dc, :],
                            start=(dc == 0), stop=(dc == DC - 1),
                        )
                    nc.scalar.activation(
                        exp_blk[:, skc, :], ps, Exp, bias=0.0, scale=scale,
                    )
                # row sums: rs[sq_blk] = sum_sk exp  (ones is a 1-col lhsT)
                ps_rs = ps_sh.tile([1, BLK], f32, name="ps_rs", tag="ps_sh")
                for skc in range(KC):
                    nc.tensor.matmul(
                        ps_rs, ones_sb, exp_blk[:, skc, :],
                        start=(skc == 0), stop=(skc == KC - 1),
                    )
                nc.vector.tensor_copy(rs_stage[:, lo:lo + BLK], ps_rs)
                # AV: U[sq, d] = sum_sk expT[sk, sq]^T V[sk, d]
                for s4 in range(SQ4):
                    sqc = blk * SQ4 + s4
                    for nb in range(ND):
                        ps = ps_av.tile([P, DB], f32, name="ps_av")
                        for skc in range(KC):
                            nc.tensor.matmul(
                                ps,
                                exp_blk[:, skc, s4 * P:(s4 + 1) * P],
                                V_sb[:, skc, nb * DB:(nb + 1) * DB],
                                start=(skc == 0), stop=(skc == KC - 1),
                            )
                        o_sb = pout.tile([P, DB], f32, name="o_sb")
                        nc.vector.tensor_copy(o_sb, ps)
                        nc.sync.dma_start(
                            out=U.ap()[sqc * P:(sqc + 1) * P, nb * DB:(nb + 1) * DB],
                            in_=o_sb,
                        )
            nc.sync.dma_start(out=rs.ap().unsqueeze(0), in_=rs_stage[0:1, :])

    nc.finalize()
    _BUILD_CACHE[key] = nc
    return nc


def _numpy_fallback(x, Wk, bk, Wq, bq, Wv, bv, dims):
    k = x @ Wk + bk
    q = x @ Wq + bq
    v = x @ Wv + bv
    s = np.einsum("bqd,bkd->bqk", q, k) / np.sqrt(np.float32(q.shape[-1]))
    s = s - s.max(axis=dims, keepdims=True)
    e = np.exp(s)
    w = e / e.sum(axis=dims, keepdims=True)
    return np.einsum("bqk,bkd->bqd", w, v).astype(np.float32)


def kernel(x, Wk, bk, Wq, bq, Wv, bv, dims):
    x = np.asarray(x, np.float32)
    Wk = np.ascontiguousarray(np.asarray(Wk, np.float32))
    Wq = np.ascontiguousarray(np.asarray(Wq, np.float32))
    Wv = np.ascontiguousarray(np.asarray(Wv, np.float32))
    bk = np.ascontiguousarray(np.asarray(bk, np.float32))
    bq = np.ascontiguousarray(np.asarray(bq, np.float32))
    bv = np.ascontiguousarray(np.asarray(bv, np.float32))
    d = int(np.asarray(dims))
    if d != 2 or x.shape != (B, S, D):
        return _numpy_fallback(x, Wk, bk, Wq, bq, Wv, bv, d)

    from concourse.bass_utils import run_bass_kernel_spmd

    nc = _build(D, S // 2, S)

    if MM == "bfloat16":
        import ml_dtypes
        cast = lambda a: np.ascontiguousarray(a.astype(ml_dtypes.bfloat16))
    else:
        cast = np.ascontiguousarray

    Wks, Wqs, Wvs = cast(Wk), cast(Wq), cast(Wv)
    half = S // 2
    in_maps = []
    for c in range(N_CORES):
        b, h = c // 2, c % 2
        xT = x[b].T  # [D, S]
        in_maps.append({
            "xkvT": cast(xT[:, h * half:(h + 1) * half]),
            "xqT": cast(xT),
            "Wk": Wks, "Wq": Wqs, "Wv": Wvs, "bk": bk, "bq": bq,
        })

    res = run_bass_kernel_spmd(nc, in_maps, core_ids=list(range(N_CORES)),
                               **_RUN_KWARGS)
    global _LAST_RESULTS
    _LAST_RESULTS = res

    out = np.empty((B, S, D), np.float32)
    for b in range(B):
        r0, r1 = res.results[2 * b], res.results[2 * b + 1]
        num = r0["U"] + r1["U"]
        den = r0["rs"] + r1["rs"]
        out[b] = num / den[:, None] + bv
    return out



# revision 4
# speedup vs baseline: 1.4212x; 1.4212x over previous
"""Fused QKV projection + correlation attention (softmax over keys) on 8 trn2 cores.

Problem: x[4,2048,1024] f32; K/Q/V = x@W* + b*; out = softmax(Q Kt / 32, keys) @ V.

Weight folding: scores = Q K^T = x (Wq Wk^T) x^T, so the host precomputes
M = Wq Wk^T (pure weight preprocessing) and the device applies M once on the
key side: GT = M xkvT.  That replaces BOTH the K and Q projections; the
score matmul consumes raw xq.  Softmax bias terms: all query-side and
constant bias terms cancel in the softmax over keys; the only survivor is
the per-key constant cb_j = x_j . (Wk bq), which the host folds (pre-scaled)
into the exp activation's per-partition bias.  bv is added on the host.

Sharding: core c -> batch b=c//2, key-half h=c%2.  Each core:
  - GT = M x_k^T for its 1024-key half; V = x_k Wv
  - scoresT[j,i] = sum_e GT[e,j] xq[i,e];  exp on ACT (scale=1/32 folded)
  - U = expT^T V (unnormalized), rs = rowsum(exp) via ones-matmul
Host combines per-batch:  out[b] = (U0+U1)/(rs0+rs1)[:,None] + bv
(no max-subtraction needed: scores ~ N(0,1), exp stays within fp32 range).

All matmul operands are bf16 (host pre-casts; on-chip evacs write bf16).
PE rate is 1 cycle/row for both bf16 and f32r; bf16 wins via half DMA bytes,
no f32r rounding copies, and everything resident in SBUF.  Accumulation is
fp32 in PSUM and exp runs on fp32 psum scores; measured rel err ~5e-3.

PE stream is interleaved across query blocks so no matmul waits on an
ACT/DVE evacuation:  GT-proj | V-proj | S0 S1 | rsAV0 | S2 | rsAV1 | S3 |
rsAV2 | rsAV3.  A single 8-bank PSUM ring matches the emission order, and
stage-0 input DMAs are issued in 512-col halves so GT-proj ramps while
M/x stream in.
"""

import numpy as np

B, S, D = 4, 2048, 1024
N_CORES = 8

_BUILD_CACHE = {}
_RUN_KWARGS = {}      # test.py sets {"trace": True, ...} for profiling runs
_LAST_RESULTS = None  # BassKernelResults of the last run


def _build(d, sk, sq):
    """Build the per-core module. d: model dim; sk: keys/core; sq: queries/core."""
    key = (d, sk, sq)
    if key in _BUILD_CACHE:
        return _BUILD_CACHE[key]

    from contextlib import ExitStack

    import concourse.bass as bass  # noqa: F401
    import concourse.mybir as mybir
    from concourse import bacc
    from concourse.tile import TileContext

    f32 = mybir.dt.float32
    bf16 = mybir.dt.bfloat16

    P = 128
    BLK = 512                # query block / psum free-dim
    DC = d // P              # d chunks (contraction + dout chunks) = 8
    KC = sk // P             # key chunks = 8
    NBLK = sq // BLK         # query blocks = 4
    SQ4 = BLK // P           # 128-row sq chunks per block = 4
    NKB = sk // BLK          # key 512-blocks = 2
    ND = d // BLK            # d 512-blocks = 2
    H = BLK                  # 512-col DMA half
    scale = float(1.0 / np.sqrt(np.float32(d)))

    nc = bacc.Bacc("TRN2", target_bir_lowering=False)
    Exp = mybir.ActivationFunctionType.Exp

    xkvT = nc.dram_tensor("xkvT", [d, sk], bf16, kind="ExternalInput")
    xqT = nc.dram_tensor("xqT", [d, sq], bf16, kind="ExternalInput")
    M = nc.dram_tensor("M", [d, d], bf16, kind="ExternalInput")
    Wv = nc.dram_tensor("Wv", [d, d], bf16, kind="ExternalInput")
    cb = nc.dram_tensor("cb", [sk], f32, kind="ExternalInput")
    U = nc.dram_tensor("U", [sq, d], f32, kind="ExternalOutput")
    rs = nc.dram_tensor("rs", [sq], f32, kind="ExternalOutput")

    xkvT_v = xkvT.ap().rearrange("(c p) s -> c p s", p=P)
    xqT_v = xqT.ap().rearrange("(c p) s -> c p s", p=P)
    M_v = M.ap().rearrange("(c p) e -> c p e", p=P)
    Wv_v = Wv.ap().rearrange("(c p) e -> c p e", p=P)

    with TileContext(nc) as tc, ExitStack() as outer:
        resid = outer.enter_context(tc.tile_pool(name="resid", bufs=1))
        psum = outer.enter_context(tc.tile_pool(name="psum", bufs=8, space="PSUM"))
        pexp = outer.enter_context(tc.tile_pool(name="pexp", bufs=2))
        pout = outer.enter_context(tc.tile_pool(name="pout", bufs=4))

        GT_sb = resid.tile([P, DC, sk], bf16)     # [d, sk]  (M-transformed keys)
        V_sb = resid.tile([P, KC, d], bf16)       # [sk, d]
        xq_sb = resid.tile([P, DC, sq], bf16)
        xkv_sb = resid.tile([P, DC, sk], bf16)
        M_sb = resid.tile([P, DC, d], bf16)
        Wv_sb = resid.tile([P, DC, d], bf16)
        cb_sb = resid.tile([P, KC], f32)
        ones_f = resid.tile([P, 1], f32)
        ones_sb = resid.tile([P, 1], bf16)
        rs_stage = resid.tile([1, sq], f32)

        nc.vector.memset(ones_f, 1.0)
        nc.vector.tensor_copy(ones_sb, ones_f)
        nc.sync.dma_start(out=cb_sb, in_=cb.ap().rearrange("(c p) -> p c", p=P))

        # ---- input DMAs, in the order compute consumes them ----------------
        # halves of M+xkv first: GT-proj groups (m<4, nb=0) start after ~2MB
        for k in range(DC):
            nc.sync.dma_start(out=xkv_sb[:, k, 0:H], in_=xkvT_v[k][:, 0:H])
            nc.sync.dma_start(out=M_sb[:, k, 0:H], in_=M_v[k][:, 0:H])
        for k in range(DC):
            nc.sync.dma_start(out=xkv_sb[:, k, H:sk], in_=xkvT_v[k][:, H:sk])
            nc.sync.dma_start(out=M_sb[:, k, H:d], in_=M_v[k][:, H:d])
        for k in range(DC):
            nc.sync.dma_start(out=Wv_sb[:, k, 0:H], in_=Wv_v[k][:, 0:H])
            nc.sync.dma_start(out=Wv_sb[:, k, H:d], in_=Wv_v[k][:, H:d])
        for blk in range(NBLK):
            for k in range(DC):
                nc.sync.dma_start(out=xq_sb[:, k, blk * BLK:(blk + 1) * BLK],
                                  in_=xqT_v[k][:, blk * BLK:(blk + 1) * BLK])

        # ---- stage 0: GT and V (key half) ----------------------------------
        def gt_group(m, nb):
            # GT[dout m, key block nb] = sum_k M[k,m]^T xkv[k,:]
            ps = psum.tile([P, BLK], f32, name="ps", tag="ps")
            for k in range(DC):
                nc.tensor.matmul(
                    ps,
                    M_sb[:, k, m * P:(m + 1) * P],
                    xkv_sb[:, k, nb * BLK:(nb + 1) * BLK],
                    start=(k == 0), stop=(k == DC - 1),
                )
            nc.scalar.copy(GT_sb[:, m, nb * BLK:(nb + 1) * BLK], ps)

        def vproj_group(m, nb):
            # V[key chunk m, d block nb] = sum_k xkv[k,m]^T Wv[k,:] (bv on host)
            ps = psum.tile([P, BLK], f32, name="ps", tag="ps")
            for k in range(DC):
                nc.tensor.matmul(
                    ps,
                    xkv_sb[:, k, m * P:(m + 1) * P],
                    Wv_sb[:, k, nb * BLK:(nb + 1) * BLK],
                    start=(k == 0), stop=(k == DC - 1),
                )
            nc.vector.tensor_copy(V_sb[:, m, nb * BLK:(nb + 1) * BLK], ps)

        # ---- stage 1 building blocks ---------------------------------------
        exp_tiles = {}

        def scores(blk):
            # expT[sk, sq_blk] = exp(scale * GT^T xq + cb)
            lo = blk * BLK
            ex = pexp.tile([P, KC, BLK], bf16, name="exp")
            exp_tiles[blk] = ex
            for skc in range(KC):
                ps = psum.tile([P, BLK], f32, name="ps", tag="ps")
                for dc in range(DC):
                    nc.tensor.matmul(
                        ps, GT_sb[:, dc, skc * P:(skc + 1) * P],
                        xq_sb[:, dc, lo:lo + BLK],
                        start=(dc == 0), stop=(dc == DC - 1),
                    )
                nc.scalar.activation(
                    ex[:, skc, :], ps, Exp,
                    bias=cb_sb[:, skc:skc + 1], scale=scale,
                )

        def rs_av(blk):
            lo = blk * BLK
            ex = exp_tiles.pop(blk)
            # row sums: rs[sq_blk] = sum_sk exp  (ones is a 1-col lhsT)
            ps_rs = psum.tile([1, BLK], f32, name="ps_rs", tag="ps")
            for skc in range(KC):
                nc.tensor.matmul(
                    ps_rs, ones_sb, ex[:, skc, :],
                    start=(skc == 0), stop=(skc == KC - 1),
                )
            nc.vector.tensor_copy(rs_stage[:, lo:lo + BLK], ps_rs)
            nc.sync.dma_start(
                out=rs.ap()[lo:lo + BLK].unsqueeze(0),
                in_=rs_stage[0:1, lo:lo + BLK],
            )
            # AV: U[sq, d] = sum_sk expT[sk, sq]^T V[sk, d]
            for s4 in range(SQ4):
                sqc = blk * SQ4 + s4
                for nb in range(ND):
                    ps = psum.tile([P, BLK], f32, name="ps", tag="ps")
                    for skc in range(KC):
                        nc.tensor.matmul(
                            ps, ex[:, skc, s4 * P:(s4 + 1) * P],
                            V_sb[:, skc, nb * BLK:(nb + 1) * BLK],
                            start=(skc == 0), stop=(skc == KC - 1),
                        )
                    o_sb = pout.tile([P, BLK], f32, name="o_sb")
                    nc.vector.tensor_copy(o_sb, ps)
                    nc.sync.dma_start(
                        out=U.ap()[sqc * P:(sqc + 1) * P, nb * BLK:(nb + 1) * BLK],
                        in_=o_sb,
                    )

        # ---- emission order == per-engine issue order ----------------------
        for m in range(DC // 2):
            gt_group(m, 0)
        for m in range(DC // 2, DC):
            gt_group(m, 0)
        for m in range(DC // 2):
            gt_group(m, 1)
        for m in range(DC // 2, DC):
            gt_group(m, 1)
        for m in range(KC):
            vproj_group(m, 0)
        for m in range(KC):
            vproj_group(m, 1)
        scores(0)
        scores(1)
        for blk in range(2, NBLK):
            rs_av(blk - 2)
            scores(blk)
        rs_av(NBLK - 2)
        rs_av(NBLK - 1)

    nc.finalize()
    _BUILD_CACHE[key] = nc
    return nc


def _numpy_fallback(x, Wk, bk, Wq, bq, Wv, bv, dims):
    k = x @ Wk + bk
    q = x @ Wq + bq
    v = x @ Wv + bv
    s = np.einsum("bqd,bkd->bqk", q, k) / np.sqrt(np.float32(q.shape[-1]))
    s = s - s.max(axis=dims, keepdims=True)
    e = np.exp(s)
    w = e / e.sum(axis=dims, keepdims=True)
    return np.einsum("bqk,bkd->bqd", w, v).astype(np.float32)


def kernel(x, Wk, bk, Wq, bq, Wv, bv, dims):
    x = np.asarray(x, np.float32)
    Wk = np.ascontiguousarray(np.asarray(Wk, np.float32))
    Wq = np.ascontiguousarray(np.asarray(Wq, np.float32))
    Wv = np.ascontiguousarray(np.asarray(Wv, np.float32))
    bk = np.ascontiguousarray(np.asarray(bk, np.float32))
    bq = np.ascontiguousarray(np.asarray(bq, np.float32))
    bv = np.ascontiguousarray(np.asarray(bv, np.float32))
    d = int(np.asarray(dims))
    if d != 2 or x.shape != (B, S, D):
        return _numpy_fallback(x, Wk, bk, Wq, bq, Wv, bv, d)

    import ml_dtypes
    from concourse.bass_utils import run_bass_kernel_spmd

    nc = _build(D, S // 2, S)

    bf = ml_dtypes.bfloat16
    cast = lambda a: np.ascontiguousarray(a.astype(bf))
    scale = np.float32(1.0 / np.sqrt(np.float32(D)))

    # weight folding (host): M = Wq Wk^T; per-key softmax bias cb = x.(Wk bq),
    # pre-scaled to match the exp activation's act(scale*psum + bias) form.
    # The device consumes M as an lhsT (computes lhsT.T @ xkvT), so pass M^T.
    Ms = cast(Wk @ Wq.T)
    Wvs = cast(Wv)
    wkbq = Wk @ bq  # [D]
    half = S // 2
    in_maps = []
    xq_cache = {}
    for c in range(N_CORES):
        b, h = c // 2, c % 2
        if b not in xq_cache:
            xq_cache[b] = cast(x[b].T)  # [D, S] bf16
        xT16 = xq_cache[b]
        cb = (scale * (x[b, h * half:(h + 1) * half] @ wkbq)).astype(np.float32)
        in_maps.append({
            "xkvT": np.ascontiguousarray(xT16[:, h * half:(h + 1) * half]),
            "xqT": xT16,
            "M": Ms, "Wv": Wvs, "cb": np.ascontiguousarray(cb),
        })

    res = run_bass_kernel_spmd(nc, in_maps, core_ids=list(range(N_CORES)),
                               **_RUN_KWARGS)
    global _LAST_RESULTS
    _LAST_RESULTS = res

    out = np.empty((B, S, D), np.float32)
    for b in range(B):
        r0, r1 = res.results[2 * b], res.results[2 * b + 1]
        num = r0["U"] + r1["U"]
        den = r0["rs"] + r1["rs"]
        out[b] = num / den[:, None] + bv
    return out


# revision 5
# speedup vs baseline: 1.6885x; 1.1880x over previous
"""Fused QKV projection + correlation attention (softmax over keys) on 8 trn2 cores.

Problem: x[4,2048,1024] f32; K/Q/V = x@W* + b*; out = softmax(Q Kt / 32, keys) @ V.

Weight folding: scores = Q K^T = x (Wq Wk^T) x^T, so the host precomputes
M = Wq Wk^T (pure weight preprocessing) and the device applies M once on the
key side: GT = M x_k^T.  That replaces BOTH the K and Q projections; the
score matmul consumes raw xq.  Softmax bias terms: all query-side and
constant bias terms cancel in the softmax over keys; the only survivor is
the per-key constant cb_j = x_j . (Wk bq), which the host folds (pre-scaled)
into the exp activation's per-partition bias.  bv is added on the host.

Sharding: core c -> batch b=c//2, key-half h=c%2.  Each core:
  - GT = M x_k^T for its 1024-key half; V = x_k Wv
  - scoresT[j,i] = sum_e GT[e,j] xq[i,e];  exp on ACT (scale=1/32 folded)
  - U = expT^T V (unnormalized), rs = rowsum(exp) via ones-matmul
Host combines per-batch:  out[b] = (U0+U1)/(rs0+rs1)[:,None] + bv
(no max-subtraction needed: scores ~ N(0,1), exp stays within fp32 range).

The host permutes each core's query columns so its key half comes first:
x_k is then just xq[:, :1024] on device (one input tensor, 2MB less DMA)
and the host un-permutes U/rs rows when combining.

All matmul operands are bf16 (host pre-casts; on-chip evacs write bf16).
PE rate is 1 cycle/row for both bf16 and f32r; bf16 wins via half DMA bytes,
no f32r rounding copies, and everything resident in SBUF.  Accumulation is
fp32 in PSUM and exp runs on fp32 psum scores; measured rel err ~7e-3.

Input DMAs move full 2-4KB contiguous dram rows (chunk-granular, so the
GT-proj k-loop paces against arriving chunks); 1KB-row slices measured only
~210GB/s (descriptor-size bound) vs ~2x for full rows.  PE stream is
interleaved across query blocks so no matmul waits on an ACT/DVE evac:
GT | V | S0 S1 | rsAV0 | S2 | rsAV1 | S3 | rsAV2 | rsAV3, with rowsum
emitted after the AV groups so the final U DMA overlaps the last matmuls.
A single 8-bank PSUM ring pool matches the emission order.
"""

import numpy as np

B, S, D = 4, 2048, 1024
N_CORES = 8

_BUILD_CACHE = {}
_RUN_KWARGS = {}      # test.py sets {"trace": True, ...} for profiling runs
_LAST_RESULTS = None  # BassKernelResults of the last run


def _build(d, sk, sq):
    """Build the per-core module. d: model dim; sk: keys/core; sq: queries/core."""
    key = (d, sk, sq)
    if key in _BUILD_CACHE:
        return _BUILD_CACHE[key]

    from contextlib import ExitStack

    import concourse.bass as bass  # noqa: F401
    import concourse.mybir as mybir
    from concourse import bacc
    from concourse.tile import TileContext

    f32 = mybir.dt.float32
    bf16 = mybir.dt.bfloat16

    P = 128
    BLK = 512                # query block / psum free-dim
    DC = d // P              # d chunks (contraction + dout chunks) = 8
    KC = sk // P             # key chunks = 8
    NBLK = sq // BLK         # query blocks = 4
    SQ4 = BLK // P           # 128-row sq chunks per block = 4
    NKB = sk // BLK          # key 512-blocks = 2
    ND = d // BLK            # d 512-blocks = 2
    scale = float(1.0 / np.sqrt(np.float32(d)))

    nc = bacc.Bacc("TRN2", target_bir_lowering=False)
    Exp = mybir.ActivationFunctionType.Exp

    xqT = nc.dram_tensor("xqT", [d, sq], bf16, kind="ExternalInput")
    M = nc.dram_tensor("M", [d, d], bf16, kind="ExternalInput")
    Wv = nc.dram_tensor("Wv", [d, d], bf16, kind="ExternalInput")
    cb = nc.dram_tensor("cb", [sk], f32, kind="ExternalInput")
    U = nc.dram_tensor("U", [sq, d], f32, kind="ExternalOutput")
    rs = nc.dram_tensor("rs", [sq], f32, kind="ExternalOutput")

    xqT_v = xqT.ap().rearrange("(c p) s -> c p s", p=P)
    M_v = M.ap().rearrange("(c p) e -> c p e", p=P)
    Wv_v = Wv.ap().rearrange("(c p) e -> c p e", p=P)

    with TileContext(nc) as tc, ExitStack() as outer:
        resid = outer.enter_context(tc.tile_pool(name="resid", bufs=1))
        psum = outer.enter_context(tc.tile_pool(name="psum", bufs=8, space="PSUM"))
        pexp = outer.enter_context(tc.tile_pool(name="pexp", bufs=2))
        pout = outer.enter_context(tc.tile_pool(name="pout", bufs=4))

        GT_sb = resid.tile([P, DC, sk], bf16)     # [d, sk]  (M-transformed keys)
        V_sb = resid.tile([P, KC, d], bf16)       # [sk, d]
        xq_sb = resid.tile([P, DC, sq], bf16)     # keys are cols [0, sk)
        M_sb = resid.tile([P, DC, d], bf16)
        Wv_sb = resid.tile([P, DC, d], bf16)
        cb_sb = resid.tile([P, KC], f32)
        ones_f = resid.tile([P, 1], f32)
        ones_sb = resid.tile([P, 1], bf16)
        rs_stage = resid.tile([1, sq], f32)

        nc.vector.memset(ones_f, 1.0)
        nc.vector.tensor_copy(ones_sb, ones_f)
        nc.sync.dma_start(out=cb_sb, in_=cb.ap().rearrange("(c p) -> p c", p=P))

        # ---- input DMAs: full contiguous dram rows, consumption order ------
        for k in range(DC):
            nc.sync.dma_start(out=M_sb[:, k, :], in_=M_v[k])
            nc.sync.dma_start(out=xq_sb[:, k, 0:sk], in_=xqT_v[k][:, 0:sk])
        for k in range(DC):
            nc.sync.dma_start(out=Wv_sb[:, k, :], in_=Wv_v[k])
            nc.sync.dma_start(out=xq_sb[:, k, sk:sq], in_=xqT_v[k][:, sk:sq])

        # ---- stage 0: GT and V (key half) ----------------------------------
        def gt_group(m, nb):
            # GT[dout m, key block nb] = sum_k (M^T)[k,m]^T xq[k, keys]
            ps = psum.tile([P, BLK], f32, name="ps", tag="ps")
            for k in range(DC):
                nc.tensor.matmul(
                    ps,
                    M_sb[:, k, m * P:(m + 1) * P],
                    xq_sb[:, k, nb * BLK:(nb + 1) * BLK],
                    start=(k == 0), stop=(k == DC - 1),
                )
            nc.scalar.copy(GT_sb[:, m, nb * BLK:(nb + 1) * BLK], ps)

        def vproj_group(m, nb):
            # V[key chunk m, d block nb] = sum_k xk[k,m]^T Wv[k,:] (bv on host)
            ps = psum.tile([P, BLK], f32, name="ps", tag="ps")
            for k in range(DC):
                nc.tensor.matmul(
                    ps,
                    xq_sb[:, k, m * P:(m + 1) * P],
                    Wv_sb[:, k, nb * BLK:(nb + 1) * BLK],
                    start=(k == 0), stop=(k == DC - 1),
                )
            nc.vector.tensor_copy(V_sb[:, m, nb * BLK:(nb + 1) * BLK], ps)

        # ---- stage 1 building blocks ---------------------------------------
        exp_tiles = {}

        def scores(blk):
            # expT[sk, sq_blk] = exp(scale * GT^T xq + cb)
            lo = blk * BLK
            ex = pexp.tile([P, KC, BLK], bf16, name="exp")
            exp_tiles[blk] = ex
            for skc in range(KC):
                ps = psum.tile([P, BLK], f32, name="ps", tag="ps")
                for dc in range(DC):
                    nc.tensor.matmul(
                        ps, GT_sb[:, dc, skc * P:(skc + 1) * P],
                        xq_sb[:, dc, lo:lo + BLK],
                        start=(dc == 0), stop=(dc == DC - 1),
                    )
                nc.scalar.activation(
                    ex[:, skc, :], ps, Exp,
                    bias=cb_sb[:, skc:skc + 1], scale=scale,
                )

        def rs_av(blk):
            lo = blk * BLK
            ex = exp_tiles.pop(blk)
            # AV: U[sq, d] = sum_sk expT[sk, sq]^T V[sk, d]
            for s4 in range(SQ4):
                sqc = blk * SQ4 + s4
                for nb in range(ND):
                    ps = psum.tile([P, BLK], f32, name="ps", tag="ps")
                    for skc in range(KC):
                        nc.tensor.matmul(
                            ps, ex[:, skc, s4 * P:(s4 + 1) * P],
                            V_sb[:, skc, nb * BLK:(nb + 1) * BLK],
                            start=(skc == 0), stop=(skc == KC - 1),
                        )
                    o_sb = pout.tile([P, BLK], f32, name="o_sb")
                    nc.vector.tensor_copy(o_sb, ps)
                    nc.sync.dma_start(
                        out=U.ap()[sqc * P:(sqc + 1) * P, nb * BLK:(nb + 1) * BLK],
                        in_=o_sb,
                    )
            # row sums last: the final U DMAs drain under these matmuls
            ps_rs = psum.tile([1, BLK], f32, name="ps_rs", tag="ps")
            for skc in range(KC):
                nc.tensor.matmul(
                    ps_rs, ones_sb, ex[:, skc, :],
                    start=(skc == 0), stop=(skc == KC - 1),
                )
            nc.vector.tensor_copy(rs_stage[:, lo:lo + BLK], ps_rs)
            nc.sync.dma_start(
                out=rs.ap()[lo:lo + BLK].unsqueeze(0),
                in_=rs_stage[0:1, lo:lo + BLK],
            )

        # ---- emission order == per-engine issue order ----------------------
        for nb in range(NKB):
            for m in range(DC):
                gt_group(m, nb)
        for nb in range(ND):
            for m in range(KC):
                vproj_group(m, nb)
        scores(0)
        scores(1)
        for blk in range(2, NBLK):
            rs_av(blk - 2)
            scores(blk)
        rs_av(NBLK - 2)
        rs_av(NBLK - 1)

    nc.finalize()
    _BUILD_CACHE[key] = nc
    return nc


def _numpy_fallback(x, Wk, bk, Wq, bq, Wv, bv, dims):
    k = x @ Wk + bk
    q = x @ Wq + bq
    v = x @ Wv + bv
    s = np.einsum("bqd,bkd->bqk", q, k) / np.sqrt(np.float32(q.shape[-1]))
    s = s - s.max(axis=dims, keepdims=True)
    e = np.exp(s)
    w = e / e.sum(axis=dims, keepdims=True)
    return np.einsum("bqk,bkd->bqd", w, v).astype(np.float32)


def kernel(x, Wk, bk, Wq, bq, Wv, bv, dims):
    x = np.asarray(x, np.float32)
    Wk = np.ascontiguousarray(np.asarray(Wk, np.float32))
    Wq = np.ascontiguousarray(np.asarray(Wq, np.float32))
    Wv = np.ascontiguousarray(np.asarray(Wv, np.float32))
    bk = np.ascontiguousarray(np.asarray(bk, np.float32))
    bq = np.ascontiguousarray(np.asarray(bq, np.float32))
    bv = np.ascontiguousarray(np.asarray(bv, np.float32))
    d = int(np.asarray(dims))
    if d != 2 or x.shape != (B, S, D):
        return _numpy_fallback(x, Wk, bk, Wq, bq, Wv, bv, d)

    import ml_dtypes
    from concourse.bass_utils import run_bass_kernel_spmd

    nc = _build(D, S // 2, S)

    bf = ml_dtypes.bfloat16
    cast = lambda a: np.ascontiguousarray(a.astype(bf))
    scale = np.float32(1.0 / np.sqrt(np.float32(D)))

    # weight folding (host): M = Wq Wk^T; per-key softmax bias cb = x.(Wk bq),
    # pre-scaled to match the exp activation's act(scale*psum + bias) form.
    # The device consumes M as an lhsT (computes lhsT.T @ xqT), so pass M^T.
    Ms = cast(Wk @ Wq.T)
    Wvs = cast(Wv)
    wkbq = Wk @ bq  # [D]
    half = S // 2
    in_maps = []
    xq_cache = {}
    for c in range(N_CORES):
        b, h = c // 2, c % 2
        if (b, h) not in xq_cache:
            xT16 = cast(x[b].T)  # [D, S] bf16
            # put the core's key half first: device reads keys at cols [0, half)
            if h == 0:
                xq_cache[(b, 0)] = xT16
            else:
                xq_cache[(b, 1)] = np.ascontiguousarray(
                    np.concatenate((xT16[:, half:], xT16[:, :half]), axis=1))
                xq_cache[(b, 0)] = xT16
        cb = (scale * (x[b, h * half:(h + 1) * half] @ wkbq)).astype(np.float32)
        in_maps.append({
            "xqT": xq_cache[(b, h)],
            "M": Ms, "Wv": Wvs, "cb": np.ascontiguousarray(cb),
        })

    res = run_bass_kernel_spmd(nc, in_maps, core_ids=list(range(N_CORES)),
                               **_RUN_KWARGS)
    global _LAST_RESULTS
    _LAST_RESULTS = res

    out = np.empty((B, S, D), np.float32)
    for b in range(B):
        r0, r1 = res.results[2 * b], res.results[2 * b + 1]
        u1, d1 = r1["U"], r1["rs"]
        # core h=1 worked in query-permuted order; un-permute its rows
        u1 = np.concatenate((u1[S // 2:], u1[:S // 2]), axis=0)
        d1 = np.concatenate((d1[S // 2:], d1[:S // 2]), axis=0)
        num = r0["U"] + u1
        den = r0["rs"] + d1
        out[b] = num / den[:, None] + bv
    return out


# revision 9
# speedup vs baseline: 1.7373x; 1.0289x over previous
"""Fused QKV projection + correlation attention (softmax over keys) on 8 trn2 cores.

Problem: x[4,2048,1024] f32; K/Q/V = x@W* + b*; out = softmax(Q Kt / 32, keys) @ V.

Weight folding: scores = Q K^T = x (Wq Wk^T) x^T, so the host precomputes
M = Wq Wk^T (pure weight preprocessing) and the device applies M once on the
key side: GT = M x_k^T.  That replaces BOTH the K and Q projections; the
score matmul consumes raw xq.  Softmax bias terms: all query-side and
constant bias terms cancel in the softmax over keys; the only survivor is
the per-key constant cb_j = x_j . (Wk bq), which the host folds (pre-scaled)
into the exp activation's per-partition bias.  bv is added on the host.

Sharding: core c -> batch b=c//2, key-half h=c%2.  Each core:
  - GT = M x_k^T for its 1024-key half; V = x_k Wv
  - scoresT[j,i] = sum_e GT[e,j] xq[i,e];  exp on ACT (scale=1/32 folded)
  - U = expT^T V (unnormalized), rs = rowsum(exp) via ones-matmul
Host combines per-batch:  out[b] = (U0+U1)/(rs0+rs1)[:,None] + bv
(no max-subtraction needed: scores ~ N(0,1), exp stays within fp32 range).

The host permutes each core's query columns so its key half comes first:
x_k is then just xq[:, :1024] on device (one input tensor, 2MB less DMA)
and the host un-permutes U/rs rows when combining.

All matmul operands are bf16 (host pre-casts; on-chip evacs write bf16).
PE rate is 1 cycle/row for both bf16 and f32r; bf16 wins via half DMA bytes,
no f32r rounding copies, and everything resident in SBUF.  Accumulation is
fp32 in PSUM and exp runs on fp32 psum scores; measured rel err ~7e-3.

Input DMAs move full 2-4KB contiguous dram rows (chunk-granular, so the
GT-proj k-loop paces against arriving chunks); 1KB-row slices measured only
~210GB/s (descriptor-size bound) vs ~2x for full rows.  PE stream is
interleaved across query blocks so no matmul waits on an ACT/DVE evac:
GT | V | S0 S1 | rsAV0 | S2 | rsAV1 | S3 | rsAV2 | rsAV3, with rowsum
emitted after the AV groups so the final U DMA overlaps the last matmuls.
A single 8-bank PSUM ring pool matches the emission order.
"""

import numpy as np

B, S, D = 4, 2048, 1024
N_CORES = 8

_BUILD_CACHE = {}
_RUN_KWARGS = {}      # test.py sets {"trace": True, ...} for profiling runs
_LAST_RESULTS = None  # BassKernelResults of the last run


def _build(d, sk, sq):
    """Build the per-core module. d: model dim; sk: keys/core; sq: queries/core."""
    key = (d, sk, sq)
    if key in _BUILD_CACHE:
        return _BUILD_CACHE[key]

    from contextlib import ExitStack

    import concourse.bass as bass  # noqa: F401
    import concourse.mybir as mybir
    from concourse import bacc
    from concourse.tile import TileContext

    f32 = mybir.dt.float32
    f32r = mybir.dt.float32r
    bf16 = mybir.dt.bfloat16

    P = 128
    BLK = 512                # query block / psum free-dim
    DC = d // P              # d chunks (contraction + dout chunks) = 8
    KC = sk // P             # key chunks = 8
    NBLK = sq // BLK         # query blocks = 4
    SQ4 = BLK // P           # 128-row sq chunks per block = 4
    NKB = sk // BLK          # key 512-blocks = 2
    ND = d // BLK            # d 512-blocks = 2
    scale = float(1.0 / np.sqrt(np.float32(d)))

    nc = bacc.Bacc("TRN2", target_bir_lowering=False)
    Exp = mybir.ActivationFunctionType.Exp

    xqT = nc.dram_tensor("xqT", [d, sq], bf16, kind="ExternalInput")
    M = nc.dram_tensor("M", [d, d], bf16, kind="ExternalInput")
    Wv = nc.dram_tensor("Wv", [d, d], bf16, kind="ExternalInput")
    cb = nc.dram_tensor("cb", [sk], f32, kind="ExternalInput")
    U = nc.dram_tensor("U", [sq, d], f32, kind="ExternalOutput")
    rs = nc.dram_tensor("rs", [sq], f32, kind="ExternalOutput")

    xqT_v = xqT.ap().rearrange("(c p) s -> c p s", p=P)
    M_v = M.ap().rearrange("(c p) e -> c p e", p=P)
    Wv_v = Wv.ap().rearrange("(c p) e -> c p e", p=P)

    with TileContext(nc) as tc, ExitStack() as outer:
        resid = outer.enter_context(tc.tile_pool(name="resid", bufs=1))
        psum = outer.enter_context(tc.tile_pool(name="psum", bufs=8, space="PSUM"))
        pexp = outer.enter_context(tc.tile_pool(name="pexp", bufs=2))
        pout = outer.enter_context(tc.tile_pool(name="pout", bufs=4))
        ptree = outer.enter_context(tc.tile_pool(name="ptree", bufs=2))

        GT_sb = resid.tile([P, DC, sk], bf16)     # [d, sk]  (M-transformed keys)
        V_sb = resid.tile([P, KC, d], bf16)       # [sk, d]
        xq_sb = resid.tile([P, DC, sq], bf16)     # keys are cols [0, sk)
        M_sb = resid.tile([P, DC, d], bf16)
        Wv_sb = resid.tile([P, DC, d], bf16)
        cb_sb = resid.tile([P, KC], f32)
        ones_f = resid.tile([P, 1], f32)
        ones_r = resid.tile([P, 1], f32r)
        rs_stage = resid.tile([1, sq], f32)

        nc.vector.memset(ones_f, 1.0)
        nc.vector.tensor_copy(ones_r, ones_f)
        nc.sync.dma_start(out=cb_sb, in_=cb.ap().rearrange("(c p) -> p c", p=P))

        # ---- input DMAs: full contiguous dram rows, consumption order ------
        for k in range(DC):
            nc.sync.dma_start(out=M_sb[:, k, :], in_=M_v[k])
            nc.sync.dma_start(out=xq_sb[:, k, 0:sk], in_=xqT_v[k][:, 0:sk])
        for k in range(DC):
            nc.sync.dma_start(out=Wv_sb[:, k, :], in_=Wv_v[k])
            nc.sync.dma_start(out=xq_sb[:, k, sk:sq], in_=xqT_v[k][:, sk:sq])

        # ---- stage 0: GT and V (key half) ----------------------------------
        def gt_group(m, nb):
            # GT[dout m, key block nb] = sum_k (M^T)[k,m]^T xq[k, keys]
            ps = psum.tile([P, BLK], f32, name="ps", tag="ps")
            for k in range(DC):
                nc.tensor.matmul(
                    ps,
                    M_sb[:, k, m * P:(m + 1) * P],
                    xq_sb[:, k, nb * BLK:(nb + 1) * BLK],
                    start=(k == 0), stop=(k == DC - 1),
                )
            nc.scalar.copy(GT_sb[:, m, nb * BLK:(nb + 1) * BLK], ps)

        def vproj_group(m, nb):
            # V[key chunk m, d block nb] = sum_k xk[k,m]^T Wv[k,:] (bv on host)
            ps = psum.tile([P, BLK], f32, name="ps", tag="ps")
            for k in range(DC):
                nc.tensor.matmul(
                    ps,
                    xq_sb[:, k, m * P:(m + 1) * P],
                    Wv_sb[:, k, nb * BLK:(nb + 1) * BLK],
                    start=(k == 0), stop=(k == DC - 1),
                )
            nc.vector.tensor_copy(V_sb[:, m, nb * BLK:(nb + 1) * BLK], ps)

        # ---- stage 1 building blocks ---------------------------------------
        exp_tiles = {}
        ar_tiles = {}

        def scores(blk):
            # expT[sk, sq_blk] = exp(scale * GT^T xq + cb)
            lo = blk * BLK
            ex = pexp.tile([P, KC, BLK], bf16, name="exp")
            exp_tiles[blk] = ex
            for skc in range(KC):
                ps = psum.tile([P, BLK], f32, name="ps", tag="ps")
                for dc in range(DC):
                    nc.tensor.matmul(
                        ps, GT_sb[:, dc, skc * P:(skc + 1) * P],
                        xq_sb[:, dc, lo:lo + BLK],
                        start=(dc == 0), stop=(dc == DC - 1),
                    )
                nc.scalar.activation(
                    ex[:, skc, :], ps, Exp,
                    bias=cb_sb[:, skc:skc + 1], scale=scale,
                )
            # DVE tree-sum of the 8 key chunks; the partition reduction then
            # needs a single ones-matmul instead of 8 (frees ~6us of PE)
            tr = ptree.tile([P, 6, BLK], f32, name="tr")
            ar = ptree.tile([P, BLK], f32r, name="ar")
            ar_tiles[blk] = ar
            for i in range(4):
                nc.vector.tensor_add(
                    tr[:, i, :], ex[:, 2 * i, :], ex[:, 2 * i + 1, :])
            nc.vector.tensor_add(tr[:, 4, :], tr[:, 0, :], tr[:, 1, :])
            nc.vector.tensor_add(tr[:, 5, :], tr[:, 2, :], tr[:, 3, :])
            nc.vector.tensor_add(ar, tr[:, 4, :], tr[:, 5, :])

        def rs_av(blk):
            lo = blk * BLK
            ex = exp_tiles.pop(blk)

            def av_group(s4, nb):
                sqc = blk * SQ4 + s4
                ps = psum.tile([P, BLK], f32, name="ps", tag="ps")
                for skc in range(KC):
                    nc.tensor.matmul(
                        ps, ex[:, skc, s4 * P:(s4 + 1) * P],
                        V_sb[:, skc, nb * BLK:(nb + 1) * BLK],
                        start=(skc == 0), stop=(skc == KC - 1),
                    )
                o_sb = pout.tile([P, BLK], f32, name="o_sb")
                nc.vector.tensor_copy(o_sb, ps)
                nc.sync.dma_start(
                    out=U.ap()[sqc * P:(sqc + 1) * P, nb * BLK:(nb + 1) * BLK],
                    in_=o_sb,
                )

            # AV: U[sq, d] = sum_sk expT[sk, sq]^T V[sk, d]; the row-sum
            # matmul goes before the last AV groups so its DMA (and the
            # trailing U DMAs) drain under PE work
            for s4 in range(SQ4 - 1):
                for nb in range(ND):
                    av_group(s4, nb)
            ps_rs = psum.tile([1, BLK], f32, name="ps_rs", tag="ps")
            nc.tensor.matmul(ps_rs, ones_r, ar_tiles.pop(blk),
                             start=True, stop=True)
            nc.vector.tensor_copy(rs_stage[:, lo:lo + BLK], ps_rs)
            nc.sync.dma_start(
                out=rs.ap()[lo:lo + BLK].unsqueeze(0),
                in_=rs_stage[0:1, lo:lo + BLK],
            )
            for nb in range(ND):
                av_group(SQ4 - 1, nb)

        # ---- emission order == per-engine issue order ----------------------
        for nb in range(NKB):
            for m in range(DC):
                gt_group(m, nb)
        for nb in range(ND):
            for m in range(KC):
                vproj_group(m, nb)
        scores(0)
        scores(1)
        for blk in range(2, NBLK):
            rs_av(blk - 2)
            scores(blk)
        rs_av(NBLK - 2)
        rs_av(NBLK - 1)

    nc.finalize()
    _BUILD_CACHE[key] = nc
    return nc


def _numpy_fallback(x, Wk, bk, Wq, bq, Wv, bv, dims):
    k = x @ Wk + bk
    q = x @ Wq + bq
    v = x @ Wv + bv
    s = np.einsum("bqd,bkd->bqk", q, k) / np.sqrt(np.float32(q.shape[-1]))
    s = s - s.max(axis=dims, keepdims=True)
    e = np.exp(s)
    w = e / e.sum(axis=dims, keepdims=True)
    return np.einsum("bqk,bkd->bqd", w, v).astype(np.float32)


def kernel(x, Wk, bk, Wq, bq, Wv, bv, dims):
    x = np.asarray(x, np.float32)
    Wk = np.ascontiguousarray(np.asarray(Wk, np.float32))
    Wq = np.ascontiguousarray(np.asarray(Wq, np.float32))
    Wv = np.ascontiguousarray(np.asarray(Wv, np.float32))
    bk = np.ascontiguousarray(np.asarray(bk, np.float32))
    bq = np.ascontiguousarray(np.asarray(bq, np.float32))
    bv = np.ascontiguousarray(np.asarray(bv, np.float32))
    d = int(np.asarray(dims))
    if d != 2 or x.shape != (B, S, D):
        return _numpy_fallback(x, Wk, bk, Wq, bq, Wv, bv, d)

    import ml_dtypes
    from concourse.bass_utils import run_bass_kernel_spmd

    nc = _build(D, S // 2, S)

    bf = ml_dtypes.bfloat16
    cast = lambda a: np.ascontiguousarray(a.astype(bf))
    scale = np.float32(1.0 / np.sqrt(np.float32(D)))

    # weight folding (host): M = Wq Wk^T; per-key softmax bias cb = x.(Wk bq),
    # pre-scaled to match the exp activation's act(scale*psum + bias) form.
    # The device consumes M as an lhsT (computes lhsT.T @ xqT), so pass M^T.
    Ms = cast(Wk @ Wq.T)
    Wvs = cast(Wv)
    wkbq = Wk @ bq  # [D]
    half = S // 2
    in_maps = []
    xq_cache = {}
    for c in range(N_CORES):
        b, h = c // 2, c % 2
        if (b, h) not in xq_cache:
            xT16 = cast(x[b].T)  # [D, S] bf16
            # put the core's key half first: device reads keys at cols [0, half)
            if h == 0:
                xq_cache[(b, 0)] = xT16
            else:
                xq_cache[(b, 1)] = np.ascontiguousarray(
                    np.concatenate((xT16[:, half:], xT16[:, :half]), axis=1))
                xq_cache[(b, 0)] = xT16
        cb = (scale * (x[b, h * half:(h + 1) * half] @ wkbq)).astype(np.float32)
        in_maps.append({
            "xqT": xq_cache[(b, h)],
            "M": Ms, "Wv": Wvs, "cb": np.ascontiguousarray(cb),
        })

    res = run_bass_kernel_spmd(nc, in_maps, core_ids=list(range(N_CORES)),
                               **_RUN_KWARGS)
    global _LAST_RESULTS
    _LAST_RESULTS = res

    out = np.empty((B, S, D), np.float32)
    for b in range(B):
        r0, r1 = res.results[2 * b], res.results[2 * b + 1]
        u1, d1 = r1["U"], r1["rs"]
        # core h=1 worked in query-permuted order; un-permute its rows
        u1 = np.concatenate((u1[S // 2:], u1[:S // 2]), axis=0)
        d1 = np.concatenate((d1[S // 2:], d1[:S // 2]), axis=0)
        num = r0["U"] + u1
        den = r0["rs"] + d1
        out[b] = num / den[:, None] + bv
    return out


# revision 13
# speedup vs baseline: 1.7744x; 1.0213x over previous
"""Fused QKV projection + correlation attention (softmax over keys) on 8 trn2 cores.

Problem: x[4,2048,1024] f32; K/Q/V = x@W* + b*; out = softmax(Q Kt / 32, keys) @ V.

Weight folding: scores = Q K^T = x (Wq Wk^T) x^T, so the host precomputes
M = Wq Wk^T (pure weight preprocessing) and the device applies M once on the
key side: GT = M x_k^T.  That replaces BOTH the K and Q projections; the
score matmul consumes raw xq.  Softmax bias terms: all query-side and
constant bias terms cancel in the softmax over keys; the only survivor is
the per-key constant cb_j = x_j . (Wk bq), which the host folds (pre-scaled)
into the exp activation's per-partition bias.  bv is added on the host.

Sharding: core c -> batch b=c//2, key-half h=c%2.  Each core:
  - GT = M x_k^T for its 1024-key half; V = x_k Wv
  - scoresT[j,i] = sum_e GT[e,j] xq[i,e];  exp on ACT (scale=1/32 folded)
  - U = expT^T V (unnormalized), rs = rowsum(exp) via ones-matmul
Host combines per-batch:  out[b] = (U0+U1)/(rs0+rs1)[:,None] + bv
(no max-subtraction needed: scores ~ N(0,1), exp stays within fp32 range).

The host permutes each core's query columns so its key half comes first:
x_k is then just xq[:, :1024] on device (one input tensor, 2MB less DMA)
and the host un-permutes U/rs rows when combining.

All matmul operands are bf16 (host pre-casts; on-chip evacs write bf16).
PE rate is 1 cycle/row for both bf16 and f32r; bf16 wins via half DMA bytes,
no f32r rounding copies, and everything resident in SBUF.  Accumulation is
fp32 in PSUM and exp runs on fp32 psum scores; measured rel err ~7e-3.

Input DMAs move full 2-4KB contiguous dram rows (chunk-granular, so the
GT-proj k-loop paces against arriving chunks); 1KB-row slices measured only
~210GB/s (descriptor-size bound) vs ~2x for full rows.  PE stream is
interleaved across query blocks so no matmul waits on an ACT/DVE evac:
GT | V | S0 S1 | rsAV0 | S2 | rsAV1 | S3 | rsAV2 | rsAV3, with rowsum
emitted after the AV groups so the final U DMA overlaps the last matmuls.
A single 8-bank PSUM ring pool matches the emission order.
"""

import numpy as np

B, S, D = 4, 2048, 1024
N_CORES = 8

_BUILD_CACHE = {}
_RUN_KWARGS = {}      # test.py sets {"trace": True, ...} for profiling runs
_LAST_RESULTS = None  # BassKernelResults of the last run


def _build(d, sk, sq):
    """Build the per-core module. d: model dim; sk: keys/core; sq: queries/core."""
    key = (d, sk, sq)
    if key in _BUILD_CACHE:
        return _BUILD_CACHE[key]

    from contextlib import ExitStack

    import concourse.bass as bass  # noqa: F401
    import concourse.mybir as mybir
    from concourse import bacc
    from concourse.tile import TileContext

    f32 = mybir.dt.float32
    f32r = mybir.dt.float32r
    bf16 = mybir.dt.bfloat16

    P = 128
    BLK = 512                # query block / psum free-dim
    DC = d // P              # d chunks (contraction + dout chunks) = 8
    KC = sk // P             # key chunks = 8
    NBLK = sq // BLK         # query blocks = 4
    SQ4 = BLK // P           # 128-row sq chunks per block = 4
    NKB = sk // BLK          # key 512-blocks = 2
    ND = d // BLK            # d 512-blocks = 2
    scale = float(1.0 / np.sqrt(np.float32(d)))

    nc = bacc.Bacc("TRN2", target_bir_lowering=False)
    Exp = mybir.ActivationFunctionType.Exp

    xqT = nc.dram_tensor("xqT", [d, sq], bf16, kind="ExternalInput")
    M = nc.dram_tensor("M", [d, d], bf16, kind="ExternalInput")
    Wv = nc.dram_tensor("Wv", [d, d], bf16, kind="ExternalInput")
    cb = nc.dram_tensor("cb", [sk], f32, kind="ExternalInput")
    U = nc.dram_tensor("U", [sq, d], f32, kind="ExternalOutput")
    rs = nc.dram_tensor("rs", [sq], f32, kind="ExternalOutput")

    xqT_v = xqT.ap().rearrange("(c p) s -> c p s", p=P)
    M_v = M.ap().rearrange("(c p) e -> c p e", p=P)
    Wv_v = Wv.ap().rearrange("(c p) e -> c p e", p=P)

    with TileContext(nc) as tc, ExitStack() as outer:
        resid = outer.enter_context(tc.tile_pool(name="resid", bufs=1))
        psum = outer.enter_context(tc.tile_pool(name="psum", bufs=8, space="PSUM"))
        pexp = outer.enter_context(tc.tile_pool(name="pexp", bufs=2))
        pout = outer.enter_context(tc.tile_pool(name="pout", bufs=4))
        ptree = outer.enter_context(tc.tile_pool(name="ptree", bufs=2))

        GT_sb = resid.tile([P, DC, sk], bf16)     # [d, sk]  (M-transformed keys)
        V_sb = resid.tile([P, KC, d], bf16)       # [sk, d]
        xq_sb = resid.tile([P, DC, sq], bf16)     # keys are cols [0, sk)
        M_sb = resid.tile([P, DC, d], bf16)
        Wv_sb = resid.tile([P, DC, d], bf16)
        cb_sb = resid.tile([P, KC], f32)
        ones_f = resid.tile([P, 1], f32)
        ones_r = resid.tile([P, 1], f32r)
        rs_stage = resid.tile([1, sq], f32)

        warm = resid.tile([P, BLK], bf16)
        nc.vector.memset(warm, 0.0)
        nc.vector.memset(ones_f, 1.0)
        nc.vector.tensor_copy(ones_r, ones_f)
        nc.sync.dma_start(out=cb_sb, in_=cb.ap().rearrange("(c p) -> p c", p=P))

        # ---- input DMAs: full contiguous dram rows, consumption order ------
        for k in range(DC):
            nc.sync.dma_start(out=M_sb[:, k, :], in_=M_v[k])
            nc.sync.dma_start(out=xq_sb[:, k, 0:sk], in_=xqT_v[k][:, 0:sk])
        for k in range(DC):
            nc.sync.dma_start(out=Wv_sb[:, k, :], in_=Wv_v[k])
            nc.sync.dma_start(out=xq_sb[:, k, sk:sq], in_=xqT_v[k][:, sk:sq])

        # ---- stage 0: GT and V (key half) ----------------------------------
        def gt_group(m, nb):
            # GT[dout m, key block nb] = sum_k (M^T)[k,m]^T xq[k, keys]
            ps = psum.tile([P, BLK], f32, name="ps", tag="ps")
            for k in range(DC):
                nc.tensor.matmul(
                    ps,
                    M_sb[:, k, m * P:(m + 1) * P],
                    xq_sb[:, k, nb * BLK:(nb + 1) * BLK],
                    start=(k == 0), stop=(k == DC - 1),
                )
            nc.scalar.copy(GT_sb[:, m, nb * BLK:(nb + 1) * BLK], ps)

        def vproj_group(m, nb):
            # V[key chunk m, d block nb] = sum_k xk[k,m]^T Wv[k,:] (bv on host)
            ps = psum.tile([P, BLK], f32, name="ps", tag="ps")
            for k in range(DC):
                nc.tensor.matmul(
                    ps,
                    xq_sb[:, k, m * P:(m + 1) * P],
                    Wv_sb[:, k, nb * BLK:(nb + 1) * BLK],
                    start=(k == 0), stop=(k == DC - 1),
                )
            nc.vector.tensor_copy(V_sb[:, m, nb * BLK:(nb + 1) * BLK], ps)

        # ---- stage 1 building blocks ---------------------------------------
        exp_tiles = {}
        ar_tiles = {}

        def scores(blk):
            # expT[sk, sq_blk] = exp(scale * GT^T xq + cb)
            lo = blk * BLK
            ex = pexp.tile([P, KC, BLK], bf16, name="exp")
            exp_tiles[blk] = ex
            for skc in range(KC):
                ps = psum.tile([P, BLK], f32, name="ps", tag="ps")
                for dc in range(DC):
                    nc.tensor.matmul(
                        ps, GT_sb[:, dc, skc * P:(skc + 1) * P],
                        xq_sb[:, dc, lo:lo + BLK],
                        start=(dc == 0), stop=(dc == DC - 1),
                    )
                nc.scalar.activation(
                    ex[:, skc, :], ps, Exp,
                    bias=cb_sb[:, skc:skc + 1], scale=scale,
                )
            # DVE tree-sum of the 8 key chunks; the partition reduction then
            # needs a single ones-matmul instead of 8 (frees ~6us of PE)
            tr = ptree.tile([P, 6, BLK], f32, name="tr")
            ar = ptree.tile([P, BLK], f32r, name="ar")
            ar_tiles[blk] = ar
            for i in range(4):
                nc.vector.tensor_add(
                    tr[:, i, :], ex[:, 2 * i, :], ex[:, 2 * i + 1, :])
            nc.vector.tensor_add(tr[:, 4, :], tr[:, 0, :], tr[:, 1, :])
            nc.vector.tensor_add(tr[:, 5, :], tr[:, 2, :], tr[:, 3, :])
            nc.vector.tensor_add(ar, tr[:, 4, :], tr[:, 5, :])

        def rs_av(blk, last=False):
            lo = blk * BLK
            ex = exp_tiles.pop(blk)

            def av_group(s4, cols, w):
                sqc = blk * SQ4 + s4
                ps = psum.tile([P, BLK], f32, name="ps", tag="ps")
                for skc in range(KC):
                    nc.tensor.matmul(
                        ps[:, 0:w], ex[:, skc, s4 * P:(s4 + 1) * P],
                        V_sb[:, skc, cols:cols + w],
                        start=(skc == 0), stop=(skc == KC - 1),
                    )
                o_sb = pout.tile([P, BLK], f32, name="o_sb")
                nc.vector.tensor_copy(o_sb[:, 0:w], ps[:, 0:w])
                nc.sync.dma_start(
                    out=U.ap()[sqc * P:(sqc + 1) * P, cols:cols + w],
                    in_=o_sb[:, 0:w],
                )

            # AV: U[sq, d] = sum_sk expT[sk, sq]^T V[sk, d]; the row-sum
            # matmul goes before the last AV groups so its DMA (and the
            # trailing U DMAs) drain under PE work
            for s4 in range(SQ4 - 1):
                for nb in range(ND):
                    av_group(s4, nb * BLK, BLK)
            ps_rs = psum.tile([1, BLK], f32, name="ps_rs", tag="ps")
            nc.tensor.matmul(ps_rs, ones_r, ar_tiles.pop(blk),
                             start=True, stop=True)
            nc.vector.tensor_copy(rs_stage[:, lo:lo + BLK], ps_rs)
            nc.sync.dma_start(
                out=rs.ap()[lo:lo + BLK].unsqueeze(0),
                in_=rs_stage[0:1, lo:lo + BLK],
            )
            # the very last groups go in quarter widths so the final U DMA
            # has less to drain after the last matmul
            w = BLK // 2 if last else BLK
            for cols in range(0, d, w):
                av_group(SQ4 - 1, cols, w)

        # ---- emission order == per-engine issue order ----------------------
        # HAM warm-up: ~5us of matmuls on zeros with no DMA dependency flips
        # the PE clock gate to 8/8 while the first input chunks are landing
        for g in range(2):
            psw = psum.tile([P, BLK], f32, name="ps", tag="ps")
            for i in range(DC):
                nc.tensor.matmul(psw, warm[:, 0:P], warm,
                                 start=(i == 0), stop=(i == DC - 1))
        for nb in range(NKB):
            for m in range(DC):
                gt_group(m, nb)
        for nb in range(ND):
            for m in range(KC):
                vproj_group(m, nb)
        scores(0)
        scores(1)
        for blk in range(2, NBLK):
            rs_av(blk - 2)
            scores(blk)
        rs_av(NBLK - 2)
        rs_av(NBLK - 1, last=True)

    nc.finalize()
    _BUILD_CACHE[key] = nc
    return nc


def _numpy_fallback(x, Wk, bk, Wq, bq, Wv, bv, dims):
    k = x @ Wk + bk
    q = x @ Wq + bq
    v = x @ Wv + bv
    s = np.einsum("bqd,bkd->bqk", q, k) / np.sqrt(np.float32(q.shape[-1]))
    s = s - s.max(axis=dims, keepdims=True)
    e = np.exp(s)
    w = e / e.sum(axis=dims, keepdims=True)
    return np.einsum("bqk,bkd->bqd", w, v).astype(np.float32)


def kernel(x, Wk, bk, Wq, bq, Wv, bv, dims):
    x = np.asarray(x, np.float32)
    Wk = np.ascontiguousarray(np.asarray(Wk, np.float32))
    Wq = np.ascontiguousarray(np.asarray(Wq, np.float32))
    Wv = np.ascontiguousarray(np.asarray(Wv, np.float32))
    bk = np.ascontiguousarray(np.asarray(bk, np.float32))
    bq = np.ascontiguousarray(np.asarray(bq, np.float32))
    bv = np.ascontiguousarray(np.asarray(bv, np.float32))
    d = int(np.asarray(dims))
    if d != 2 or x.shape != (B, S, D):
        return _numpy_fallback(x, Wk, bk, Wq, bq, Wv, bv, d)

    import ml_dtypes
    from concourse.bass_utils import run_bass_kernel_spmd

    nc = _build(D, S // 2, S)

    bf = ml_dtypes.bfloat16
    cast = lambda a: np.ascontiguousarray(a.astype(bf))
    scale = np.float32(1.0 / np.sqrt(np.float32(D)))

    # weight folding (host): M = Wq Wk^T; per-key softmax bias cb = x.(Wk bq),
    # pre-scaled to match the exp activation's act(scale*psum + bias) form.
    # The device consumes M as an lhsT (computes lhsT.T @ xqT), so pass M^T.
    Ms = cast(Wk @ Wq.T)
    Wvs = cast(Wv)
    wkbq = Wk @ bq  # [D]
    half = S // 2
    in_maps = []
    xq_cache = {}
    for c in range(N_CORES):
        b, h = c // 2, c % 2
        if (b, h) not in xq_cache:
            xT16 = cast(x[b].T)  # [D, S] bf16
            # put the core's key half first: device reads keys at cols [0, half)
            if h == 0:
                xq_cache[(b, 0)] = xT16
            else:
                xq_cache[(b, 1)] = np.ascontiguousarray(
                    np.concatenate((xT16[:, half:], xT16[:, :half]), axis=1))
                xq_cache[(b, 0)] = xT16
        cb = (scale * (x[b, h * half:(h + 1) * half] @ wkbq)).astype(np.float32)
        in_maps.append({
            "xqT": xq_cache[(b, h)],
            "M": Ms, "Wv": Wvs, "cb": np.ascontiguousarray(cb),
        })

    res = run_bass_kernel_spmd(nc, in_maps, core_ids=list(range(N_CORES)),
                               **_RUN_KWARGS)
    global _LAST_RESULTS
    _LAST_RESULTS = res

    out = np.empty((B, S, D), np.float32)
    for b in range(B):
        r0, r1 = res.results[2 * b], res.results[2 * b + 1]
        u1, d1 = r1["U"], r1["rs"]
        # core h=1 worked in query-permuted order; un-permute its rows
        u1 = np.concatenate((u1[S // 2:], u1[:S // 2]), axis=0)
        d1 = np.concatenate((d1[S // 2:], d1[:S // 2]), axis=0)
        num = r0["U"] + u1
        den = r0["rs"] + d1
        out[b] = num / den[:, None] + bv
    return out
